# revision 25
# baseline (speedup 1.0000x reference)
"""RGCN (EntityClassifyHeteroAPI) Trainium2 kernel: 3-layer hetero message passing.

v2 strategy (8 NeuronCores, SPMD single program):
  - Shard destination nodes: core c owns dst rows [c*6250, (c+1)*6250).
  - Layer 0 messages (embed[src]) are pre-expanded on HOST into a per-core
    chunked stream loaded with static DMA -- no on-device gathers in l0.
  - Layers 1/2 gather from the replicated bf16 node table via
    gpsimd.dma_gather.  The table is SPLIT into two DRAM tensors at phys
    row 32768 (h_A / h_B) so int16 indices work AND the AllGather can run
    in two phases with gathers from h_A starting before h_B arrives.
  - Slot-granular bucket packing: per (block, half, relation) groups are
    padded only to the cross-core max EDGE COUNT (not to 128-chunk
    multiples); chunk boundaries fall wherever they fall and each
    (b, r, chunk) intersection gets its own one-hot segment column.
    ~78k gathered rows/layer vs ~115k with chunk-granular padding.
  - Aggregation per (block, relation) via one-hot matmuls accumulating in
    PSUM (feat-major aggT); per-relation GEMM against W quadrants; bias+
    relu; stores feed the 2-phase AllGather (A = blocks 0..31 = phys rows
    [0, 32768), B = blocks 32..48).
  - Gather issue order interleaves: lo-half gathers lead hi-half gathers
    by LAG pairs, so the hi gathers' wait on AllGather-B is covered by lo
    gather work and GPSIMD never idles on the collective.
  - Host precomputes the whole schedule from src/dst; the Bass program is
    identical across cores (all counts are cross-core maxima).
"""
import os
import sys

import numpy as np

for _p in ("/opt/trn_rl_repo", "/root/.axon_site/_ro/trn_rl_repo"):
    if _p not in sys.path and os.path.isdir(_p):
        sys.path.insert(0, _p)

import ml_dtypes  # noqa: E402
import concourse.bacc as bacc  # noqa: E402
import concourse.bass as bass  # noqa: E402
import concourse.mybir as mybir  # noqa: E402
import concourse.tile as tile  # noqa: E402
from concourse import bass_utils  # noqa: E402

N_NODES = 50000
H = 256
O = 64
R = 8
E_PER_R = 65536
NCORES = 8
NSH = N_NODES // NCORES  # 6250
BLK = 128                # dst nodes per aggregation block
NB = (NSH + BLK - 1) // BLK  # 49 blocks (last has 106 nodes)
NBP = (NB + 1) // 2          # 25 block pairs

# Table split: the A-table (phys rows [0, SPLIT)) holds the LATE-computed
# blocks 17-48 and is gathered LAGGING; the B-table (phys [SPLIT, 50000))
# holds the EARLY blocks 0-16 and is gathered LEADING.  This way every
# region's AllGather fires right after its blocks are stored, and each
# table is complete before the next layer's gathers need it.
SPLIT = NCORES * (1536 + 1536 + 1002)   # 32592 rows in the A-table
HA_ROWS = SPLIT
HB_ROWS = N_NODES - SPLIT               # 17408

# Collective phases per layer boundary; each is an AllGather of one
# contiguous per-core slice (jl range) into a contiguous phys-row range.
# (jl0, jl1, phys_base, hook_pair): hook_pair = consume-pair after which
# all blocks in the region have been stored.
REGIONS = [
    (2176, 4224, 0, 16),      # A1: blocks 17-32 (front-heavy)
    (4224, 5504, 16384, 21),  # A2: blocks 33-42
    (5504, 6144, 26624, 23),  # A3: blocks 43-47
    (6144, 6250, 31744, 24),  # A4: block 48 (tiny tail, hook deferred)
    (0, 1024, 32592, 3),      # B1: blocks 0-7
    (1024, 2176, 40784, 8),   # B2: blocks 8-16
]
NREG = len(REGIONS)
# in-loop emission consume-index per region (store-wait slack baked in);
# A4 (index 3) is deferred into the NEXT layer's gather loop, emitted
# right before the first lagged (A-table) gather.
HOOK_EMIT = {0: 16, 1: 21, 2: 24, 4: 5, 5: 10}
HOOK_EMIT0 = {0: 17, 1: 22, 2: 24, 4: 4, 5: 9}

BF16 = ml_dtypes.bfloat16

LAG = int(os.environ.get("BASS_GNN_LAG", "8"))
CONS = int(os.environ.get("BASS_GNN_CONS", "1"))
CONS0 = int(os.environ.get("BASS_GNN_CONS0", "2"))
SCRATCH = int(os.environ.get("BASS_GNN_SCRATCH", "24576"))


def _phys_row(j):
    c, jl = j // NSH, j % NSH
    out = np.zeros_like(np.asarray(j))
    for jl0, jl1, base, _ in REGIONS:
        sel = (jl >= jl0) & (jl < jl1)
        out = np.where(sel, base + c * (jl1 - jl0) + (jl - jl0), out)
    return out


def _ceil_div(a, b):
    return -(-a // b)


def _wrap_idx(gidx):
    """[S] int16 -> [128, S//16] (16-partition wrap, replicated x8)."""
    w = gidx.reshape(-1, 16).T
    return np.ascontiguousarray(np.tile(w, (8, 1)))


def _preprocess(src, dst, embed):
    """Build the SPMD schedule + per-core input arrays.

    Returns (sched, percore) where sched holds cross-core-identical
    metadata and percore[c] the per-core gidx/offs/msg0 arrays.
    """
    src_o = np.asarray(src).reshape(-1).astype(np.int64)
    src_f = _phys_row(src_o)
    dst_f = np.asarray(dst).reshape(-1).astype(np.int64)
    rel_f = np.repeat(np.arange(R, dtype=np.int64), E_PER_R)

    core = dst_f // NSH
    dloc = dst_f - core * NSH
    b_of = dloc // BLK
    nloc = dloc - b_of * BLK
    half = (src_f >= SPLIT).astype(np.int64)

    # ---------------- layers 1/2 schedule ----------------
    key = ((core * NB + b_of) * 2 + half) * R + rel_f
    cnt = np.bincount(key, minlength=NCORES * NB * 2 * R).reshape(
        NCORES, NB, 2, R)
    W12 = cnt.max(axis=0)  # [NB, 2, R] slot widths (cross-core max)

    pair_meta = []   # per bp: dict(nseg, nch=[lo,hi], segbase, cmap, blks)
    slot0 = {}       # (b, hf, r) -> slot offset within its pair-half
    segidx = {}      # (b, hf, r, ci_local) -> seg col local to pair
    seg_global = 0
    for bp in range(NBP):
        blks = [b for b in (2 * bp, 2 * bp + 1) if b < NB]
        cmap = {b: {r: [] for r in range(R)} for b in blks}
        nseg_local = 0
        nch = [0, 0]
        for hf in (0, 1):
            cur = 0
            for b in blks:
                for r in range(R):
                    w = int(W12[b, hf, r])
                    if w == 0:
                        continue
                    slot0[(b, hf, r)] = cur
                    for ci in range(cur // 128, (cur + w - 1) // 128 + 1):
                        cmap[b][r].append((hf, ci, nseg_local))
                        segidx[(b, hf, r, ci)] = nseg_local
                        nseg_local += 1
                    cur += w
            nch[hf] = _ceil_div(cur, 128)
        pair_meta.append(dict(blks=blks, cmap=cmap, nseg=nseg_local,
                              nch=nch, segbase=seg_global))
        seg_global += nseg_local
    S12 = seg_global
    base12 = {}
    g = 0
    for bp in range(NBP):
        for hf in (0, 1):
            base12[(bp, hf)] = g
            g += pair_meta[bp]['nch'][hf]
    C12 = g
    MAXLO = max(m['nch'][0] for m in pair_meta)
    MAXHI = max(m['nch'][1] for m in pair_meta)
    MAXSEG = max(m['nseg'] for m in pair_meta)

    # ---------------- layer 0 schedule ----------------
    key0 = core * NB + b_of
    cnt0 = np.bincount(key0, minlength=NCORES * NB).reshape(NCORES, NB)
    W0 = cnt0.max(axis=0)
    ch0 = _ceil_div(W0, 128)          # chunks per block (block-aligned)
    cb0 = np.concatenate([[0], np.cumsum(ch0)])  # global chunk base per blk
    C0 = int(cb0[-1])
    MAXCH0 = max(int(ch0[2 * bp] + ch0[2 * bp + 1]) if 2 * bp + 1 < NB
                 else int(ch0[2 * bp]) for bp in range(NBP))

    sched = dict(W12=W12, pair_meta=pair_meta, base12=base12, C12=C12,
                 S12=S12, MAXLO=MAXLO, MAXHI=MAXHI, MAXSEG=MAXSEG,
                 W0=W0, ch0=ch0, cb0=cb0, C0=C0, MAXCH0=MAXCH0)

    # ---------------- per-core arrays ----------------
    emb_bf = np.asarray(embed, dtype=np.float32).astype(BF16)
    # slot offset of (b, hf, r) inside the pair-half, as flat arrays
    slot0_arr = np.zeros((NB, 2, R), np.int64)
    for (b, hf, r), s in slot0.items():
        slot0_arr[b, hf, r] = s
    base12_arr = np.zeros((NBP, 2), np.int64)
    for (bp, hf), v in base12.items():
        base12_arr[bp, hf] = v

    percore = []
    for c in range(NCORES):
        m = core == c
        sf = src_f[m]
        so = src_o[m]
        rf = rel_f[m]
        bf = b_of[m]
        nf = nloc[m]
        hf_e = half[m]

        # --- l12: per-edge slots ---
        gid = (bf * 2 + hf_e) * R + rf   # group id (b, hf, r)
        order = np.argsort(gid, kind='stable')
        gid_s = gid[order]
        starts = np.concatenate(
            [[0], np.cumsum(np.bincount(gid_s, minlength=NB * 2 * R))])
        pos = np.arange(gid_s.size) - starts[gid_s]
        b_s, hf_s, r_s = bf[order], hf_e[order], rf[order]
        sf_s, nf_s = sf[order], nf[order]
        slot_ph = slot0_arr[b_s, hf_s, r_s] + pos      # slot in pair-half
        assert (pos < W12[b_s, hf_s, r_s]).all(), "slot overflow"
        bp_s = b_s // 2
        gcol = base12_arr[bp_s, hf_s] + slot_ph // 128  # global chunk
        e_in = slot_ph % 128
        gidx = np.zeros(C12 * 128, np.int16)
        gidx[gcol * 128 + e_in] = (sf_s - hf_s * SPLIT).astype(np.int16)
        offs12 = np.full((128, S12), -1.0, np.float32)
        segb = np.array([pair_meta[bp]['segbase'] for bp in range(NBP)])
        ci_l = slot_ph // 128
        segcol = np.array(
            [segidx[(b, h, r, ci)]
             for b, h, r, ci in zip(b_s, hf_s, r_s, ci_l)], np.int64)
        offs12[e_in, segb[bp_s] + segcol] = nf_s

        # --- l0: per-edge slots + host-expanded messages ---
        order0 = np.argsort(bf, kind='stable')
        b0 = bf[order0]
        starts0 = np.concatenate(
            [[0], np.cumsum(np.bincount(b0, minlength=NB))])
        pos0 = np.arange(b0.size) - starts0[b0]
        assert (pos0 < W0[b0]).all()
        ch = cb0[b0] + pos0 // 128
        e0 = pos0 % 128
        msg0 = np.zeros((128, C0, H), BF16)
        msg0[e0, ch, :] = emb_bf[so[order0]]
        offs0 = np.full((128, C0), -1.0, np.float32)
        offs0[e0, ch] = nf[order0]

        percore.append(dict(gidx12=_wrap_idx(gidx), offs12=offs12,
                            msg0=np.ascontiguousarray(
                                msg0.reshape(128, C0 * H)),
                            offs0=offs0))

    return sched, percore


def _build_program(sched, b0z, b1z, b2z):
    """Build the SPMD Bass program (same for all cores)."""
    nc = bacc.Bacc(None, target_bir_lowering=False, debug=False,
                   num_swdge_queues=4, dynamic_dma_scratch_size=SCRATCH)
    f32, bf16, i16 = mybir.dt.float32, mybir.dt.bfloat16, mybir.dt.int16

    pair_meta = sched['pair_meta']
    base12 = sched['base12']
    C12, S12, C0 = sched['C12'], sched['S12'], sched['C0']
    MAXLO, MAXHI = sched['MAXLO'], sched['MAXHI']
    MAXSEG, MAXCH0 = sched['MAXSEG'], sched['MAXCH0']
    ch0, cb0 = sched['ch0'], sched['cb0']

    w1 = nc.dram_tensor("w1", [R, H, H], bf16, kind="ExternalInput")
    w2 = nc.dram_tensor("w2", [R, H, O], bf16, kind="ExternalInput")
    b0r = nc.dram_tensor("b0r", [128, H], f32, kind="ExternalInput")
    b1r = nc.dram_tensor("b1r", [128, H], f32, kind="ExternalInput")
    b2r = nc.dram_tensor("b2r", [128, O], f32, kind="ExternalInput")
    gidx12_d = nc.dram_tensor("gidx12", [128, C12 * 8], i16,
                              kind="ExternalInput")
    offs12_d = nc.dram_tensor("offs12", [128, S12], f32,
                              kind="ExternalInput")
    offs0_d = nc.dram_tensor("offs0", [128, C0], f32, kind="ExternalInput")
    msg0_d = nc.dram_tensor("msg0", [128, C0 * H], bf16,
                            kind="ExternalInput")
    iota_d = nc.dram_tensor("iotaf", [128, BLK], f32, kind="ExternalInput")
    out_d = nc.dram_tensor("out", [NSH, O], f32, kind="ExternalOutput")

    # per-layer-boundary staging: one input tensor per collective region
    ag_in = {}
    for li in (0, 1):
        for ri, (jl0, jl1, base, hook) in enumerate(REGIONS):
            ag_in[(li, ri)] = nc.dram_tensor(
                f"ag{li}_in{ri}", [jl1 - jl0, H], bf16)
    h0A = nc.dram_tensor("h0A", [HA_ROWS, H], bf16, addr_space="Shared")
    h0B = nc.dram_tensor("h0B", [HB_ROWS, H], bf16, addr_space="Shared")
    h1A = nc.dram_tensor("h1A", [HA_ROWS, H], bf16, addr_space="Shared")
    h1B = nc.dram_tensor("h1B", [HB_ROWS, H], bf16, addr_space="Shared")

    # independent mod-4 counters for lo/hi gather calls so every queue sees
    # the same mix of large (lo) and small (hi) calls
    _qlo, _qhi = [0], [2]

    def next_q(ctr):
        q = ctr[0]
        ctr[0] = (ctr[0] + 1) % 4
        return q

    with tile.TileContext(nc) as tc:
        with (
            tc.tile_pool(name="const", bufs=1) as constp,
            tc.tile_pool(name="wpool", bufs=1) as wpool,
            tc.tile_pool(name="agg", bufs=3) as aggp,
            tc.tile_pool(name="hout", bufs=4) as houtp,
            tc.tile_pool(name="psagg", bufs=6, space="PSUM") as psaggp,
            tc.tile_pool(name="psh", bufs=2, space="PSUM") as pshp,
        ):
            # resident consts; l0's tables on sync first so l0 starts
            # immediately; the big l1/l2 index tables go on the Act HWDGE
            # queue so they don't delay the msg0 stream.
            offs0_sb = constp.tile([128, C0], f32, tag="of0")
            nc.sync.dma_start(out=offs0_sb[:], in_=offs0_d[:])
            iota_sb = constp.tile([128, BLK], f32, tag="iota")
            nc.sync.dma_start(out=iota_sb[:], in_=iota_d[:])
            gidx12_sb = constp.tile([128, C12 * 8], i16, tag="g12")
            nc.scalar.dma_start(out=gidx12_sb[:], in_=gidx12_d[:])
            offs12_sb = constp.tile([128, S12], f32, tag="of12")
            nc.scalar.dma_start(out=offs12_sb[:], in_=offs12_d[:])

            bias_sb = {}
            for name, t, width, z in (
                ("b0", b0r, H, b0z), ("b1", b1r, H, b1z),
                ("b2", b2r, O, b2z),
            ):
                if not z:
                    bias_sb[name] = constp.tile([128, width], f32, tag=name)
                    nc.scalar.dma_start(out=bias_sb[name][:], in_=t[:])

            # resident W tiles [r][fh] = [128, hout_w] bf16 (both layers)
            wt = {}
            for wname, w_d, hout_w in (("w1", w1, H), ("w2", w2, O)):
                rows = []
                for r in range(R):
                    row = []
                    for fh in range(2):
                        t = wpool.tile([128, hout_w], bf16,
                                       tag=f"{wname}_{r}_{fh}")
                        nc.scalar.dma_start(
                            out=t[:], in_=w_d[r, fh * 128:(fh + 1) * 128, :])
                        row.append(t)
                    rows.append(row)
                wt[wname] = rows

            def ag_region(li, ri, h_a, h_b):
                """AllGather collective for region ri of layer li's table."""
                jl0, jl1, base, _ = REGIONS[ri]
                nrows = (jl1 - jl0) * NCORES
                if base < SPLIT:
                    out_ap = h_a[base:base + nrows, :]
                else:
                    out_ap = h_b[base - SPLIT:base - SPLIT + nrows, :]

                def go():
                    nc.gpsimd.collective_compute(
                        "AllGather", mybir.AluOpType.bypass,
                        ins=[ag_in[(li, ri)][:]], outs=[out_ap],
                        replica_groups=[list(range(NCORES))],
                    )
                return go

            def ag_store(li, b, nrows, src_ap):
                row0 = b * BLK
                for ri, (jl0, jl1, base, _) in enumerate(REGIONS):
                    if jl0 <= row0 < jl1:
                        nc.sync.dma_start(
                            out=ag_in[(li, ri)][row0 - jl0:
                                                row0 - jl0 + nrows, :],
                            in_=src_ap)
                        return
                raise AssertionError(b)

            def store_h(li):
                def go(b, nrows, psh):
                    hsb = houtp.tile([128, H], bf16, tag="hsb")
                    if "b1" in bias_sb:
                        tmp = houtp.tile([128, H], f32, tag="htmp")
                        nc.vector.tensor_tensor(
                            out=tmp[:], in0=psh[:], in1=bias_sb["b1"][:],
                            op=mybir.AluOpType.add)
                        nc.scalar.activation(
                            hsb[:], tmp[:], mybir.ActivationFunctionType.Relu)
                    else:
                        nc.scalar.activation(
                            hsb[:], psh[:], mybir.ActivationFunctionType.Relu)
                    ag_store(li, b, nrows, hsb[:nrows, :])
                return go

            def store_out(b, nrows, psh):
                osb = houtp.tile([128, O], f32, tag="osb")
                if "b2" in bias_sb:
                    nc.vector.tensor_tensor(
                        out=osb[:], in0=psh[:], in1=bias_sb["b2"][:],
                        op=mybir.AluOpType.add)
                else:
                    nc.vector.tensor_copy(out=osb[:], in_=psh[:])
                nc.sync.dma_start(
                    out=out_d[b * BLK:b * BLK + nrows, :],
                    in_=osb[:nrows, :])

            # ---------------- layer 0 ----------------
            with (
                tc.tile_pool(name="msg0", bufs=CONS0 + 2) as msg0p,
                tc.tile_pool(name="oh0", bufs=CONS0 + 2) as oh0p,
            ):
                def load0(p):
                    b0 = 2 * p
                    nch = int(ch0[b0]) + (int(ch0[b0 + 1])
                                          if b0 + 1 < NB else 0)
                    col0 = int(cb0[b0])
                    t = msg0p.tile([128, MAXCH0, H], bf16, tag="m0")
                    eng = nc.sync if p % 2 == 0 else nc.scalar
                    eng.dma_start(
                        out=t[:, :nch, :],
                        in_=msg0_d[:, col0 * H:(col0 + nch) * H])
                    return t

                def oh0gen(p):
                    b0 = 2 * p
                    nch = int(ch0[b0]) + (int(ch0[b0 + 1])
                                          if b0 + 1 < NB else 0)
                    col0 = int(cb0[b0])
                    oh = oh0p.tile([128, MAXCH0, BLK], bf16, tag="oh0")
                    nc.vector.tensor_tensor(
                        out=oh[:, :nch, :],
                        in0=iota_sb[:].unsqueeze(1)
                            .broadcast_to([128, nch, BLK]),
                        in1=offs0_sb[:, col0:col0 + nch].unsqueeze(2)
                            .broadcast_to([128, nch, BLK]),
                        op=mybir.AluOpType.is_equal)
                    return oh

                def consume0(p, msg, oh):
                    b0 = 2 * p
                    for b in (b0, b0 + 1):
                        if b >= NB:
                            continue
                        nrows = min(BLK, NSH - b * BLK)
                        nch = int(ch0[b])
                        c_l = int(cb0[b] - cb0[b0])
                        ps = psaggp.tile([128, H], f32, space="PSUM",
                                         tag="ps")
                        for i in range(nch):
                            nc.tensor.matmul(
                                ps[:],
                                lhsT=oh[:, c_l + i, :],
                                rhs=msg[:, c_l + i, :],
                                start=(i == 0), stop=(i == nch - 1))
                        hsb = houtp.tile([128, H], bf16, tag="h0sb")
                        if "b0" in bias_sb:
                            tmp = houtp.tile([128, H], f32, tag="h0tmp")
                            nc.vector.tensor_tensor(
                                out=tmp[:], in0=ps[:], in1=bias_sb["b0"][:],
                                op=mybir.AluOpType.add)
                            nc.scalar.activation(
                                hsb[:], tmp[:],
                                mybir.ActivationFunctionType.Relu)
                        else:
                            nc.scalar.activation(
                                hsb[:], ps[:],
                                mybir.ActivationFunctionType.Relu)
                        ag_store(0, b, nrows, hsb[:nrows, :])

                # emission slack so the AG's store-wait is already
                # satisfied when gpsimd reaches it; A3 is deferred into
                # layer 1's gather loop.
                hooks0 = {ei: ag_region(0, ri, h0A, h0B)
                          for ri, ei in HOOK_EMIT0.items()}
                msgs, ohs = {}, {}
                for k in range(NBP + CONS0):
                    if k < NBP:
                        msgs[k] = load0(k)
                        ohs[k] = oh0gen(k)
                    i = k - CONS0
                    if 0 <= i < NBP:
                        consume0(i, msgs.pop(i), ohs.pop(i))
                        if i in hooks0:
                            hooks0[i]()

            # ---------------- layers 1 and 2 ----------------
            with (
                tc.tile_pool(name="mlo", bufs=2 * CONS + 3) as mlop,
                tc.tile_pool(name="mhi", bufs=LAG + CONS + 4) as mhip,
                tc.tile_pool(name="ohp", bufs=2 * CONS + 1) as ohp,
            ):
                def run_layer(h_a, h_b, wname, hout_w, store, hooks,
                              pre_hooks):
                    def glo(p):
                        nch = pair_meta[p]['nch'][0]
                        t = mlop.tile([128, MAXLO, H], bf16, tag="mlo")
                        if nch:
                            c0 = base12[(p, 0)]
                            nidx = nch * 128
                            nc.gpsimd.dma_gather(
                                t[:, :nch, :], h_a[:],
                                gidx12_sb[:, c0 * 8:(c0 + nch) * 8],
                                nidx, nidx, H,
                                queue_num=next_q(_qlo), single_packet=False)
                        return t

                    def ghi(p):
                        nch = pair_meta[p]['nch'][1]
                        t = mhip.tile([128, MAXHI, H], bf16, tag="mhi")
                        if nch:
                            c0 = base12[(p, 1)]
                            nidx = nch * 128
                            nc.gpsimd.dma_gather(
                                t[:, :nch, :], h_b[:],
                                gidx12_sb[:, c0 * 8:(c0 + nch) * 8],
                                nidx, nidx, H,
                                queue_num=next_q(_qhi), single_packet=False)
                        return t

                    def ohgen(p):
                        ns = pair_meta[p]['nseg']
                        sb = pair_meta[p]['segbase']
                        oh = ohp.tile([128, MAXSEG, BLK], bf16, tag="oh")
                        nc.vector.tensor_tensor(
                            out=oh[:, :ns, :],
                            in0=iota_sb[:].unsqueeze(1)
                                .broadcast_to([128, ns, BLK]),
                            in1=offs12_sb[:, sb:sb + ns].unsqueeze(2)
                                .broadcast_to([128, ns, BLK]),
                            op=mybir.AluOpType.is_equal)
                        return oh

                    def consume(p, mlo, mhi, oh):
                        meta = pair_meta[p]
                        for b in meta['blks']:
                            nrows = min(BLK, NSH - b * BLK)
                            live = [r for r in range(R) if meta['cmap'][b][r]]
                            psh = pshp.tile([128, hout_w], f32, space="PSUM",
                                            tag="psh")
                            aggs = {}
                            for r in live:
                                segs = meta['cmap'][b][r]
                                ps = psaggp.tile([128, H], f32, space="PSUM",
                                                 tag="ps")
                                last = len(segs) - 1
                                for i, (hf, ci, sg) in enumerate(segs):
                                    m = mlo if hf == 0 else mhi
                                    for fh in range(2):
                                        nc.tensor.matmul(
                                            ps[:, fh * BLK:(fh + 1) * BLK],
                                            lhsT=m[:, ci,
                                                   fh * 128:(fh + 1) * 128],
                                            rhs=oh[:, sg, :],
                                            start=(i == 0 and fh == 0),
                                            stop=(i == last))
                                agg_sb = aggp.tile([128, H], bf16,
                                                   tag=f"agg{r}")
                                if r % 2 == 0:
                                    nc.scalar.activation(
                                        agg_sb[:], ps[:],
                                        mybir.ActivationFunctionType.Copy)
                                else:
                                    nc.vector.tensor_copy(
                                        out=agg_sb[:], in_=ps[:])
                                aggs[r] = agg_sb
                            for ri, r in enumerate(live):
                                for fh in range(2):
                                    nc.tensor.matmul(
                                        psh[:],
                                        lhsT=aggs[r][:,
                                                     fh * BLK:(fh + 1) * BLK],
                                        rhs=wt[wname][r][fh][:],
                                        start=(ri == 0 and fh == 0),
                                        stop=(ri == len(live) - 1
                                              and fh == 1))
                            store(b, nrows, psh)

                    # Catch-up schedule: the leading (B-table) stream runs
                    # 1 pair/iter; once the lagged (A-table) stream starts it
                    # issues 2 pairs/iter until it overtakes, so the last
                    # pairs' lagged gathers are not bunched at the tail.
                    # consume follows min(lead, lag) - CONS at up to 2/iter.
                    mlos, mhis, ohs_ = {}, {}, {}
                    gl = cons = 0
                    for k in range(NBP + LAG + CONS):
                        if k < NBP:
                            mhis[k] = ghi(k)
                        if k in pre_hooks:
                            pre_hooks[k]()
                        if k >= LAG:
                            for _ in range(2):
                                if gl < min(NBP, 2 * (k - LAG + 1)):
                                    mlos[gl] = glo(gl)
                                    ohs_[gl] = ohgen(gl)
                                    gl += 1
                        lim = gl if k >= NBP - 1 else min(gl, k + 1) - CONS
                        while cons < lim:
                            i = cons
                            consume(i, mlos.pop(i), mhis.pop(i),
                                    ohs_.pop(i))
                            if i in hooks:
                                hooks[i]()
                            cons += 1
                        if cons >= NBP:
                            break

                hooks1 = {ei: ag_region(1, ri, h1A, h1B)
                          for ri, ei in HOOK_EMIT.items()}
                pre1 = {LAG: ag_region(0, 3, h0A, h0B)}   # l0's A4
                pre2 = {LAG: ag_region(1, 3, h1A, h1B)}   # l1's A4
                run_layer(h0A, h0B, "w1", H, store_h(1), hooks1, pre1)
                run_layer(h1A, h1B, "w2", O, store_out, {}, pre2)

    nc.finalize()
    return nc


def _install_ntff_shim():
    """Provide antenv.axon_hooks (missing in this image) so trace=True works."""
    import types
    try:
        from antenv.axon_hooks import get_axon_ntff_profile_hook  # noqa: F401
        return
    except ImportError:
        pass
    mod = types.ModuleType("antenv.axon_hooks")
    state = {"hook": None}
    mod.set_axon_ntff_profile_hook = lambda h: state.__setitem__("hook", h)
    mod.get_axon_ntff_profile_hook = lambda: state["hook"]
    try:
        import antenv
        antenv.axon_hooks = mod
    except ImportError:
        pkg = types.ModuleType("antenv")
        pkg.axon_hooks = mod
        sys.modules["antenv"] = pkg
    sys.modules["antenv.axon_hooks"] = mod
    try:
        from trn_agent_boot.trn_boot import _ntff_profile_via_ctypes
        hook = _ntff_profile_via_ctypes("/opt/axon/libaxon_pjrt.so")
        mod.set_axon_ntff_profile_hook(hook)
    except Exception as e:
        print(f"[kernel] ntff shim failed: {e}", file=sys.stderr)


def kernel(embed, src, dst, W1, b0, b1, W2, b2):
    embed = np.asarray(embed, dtype=np.float32)
    W1 = np.asarray(W1, dtype=np.float32)
    W2 = np.asarray(W2, dtype=np.float32)
    b0 = np.asarray(b0, dtype=np.float32)
    b1 = np.asarray(b1, dtype=np.float32)
    b2 = np.asarray(b2, dtype=np.float32)

    sched, percore = _preprocess(src, dst, embed)

    nc = _build_program(
        sched,
        bool(np.all(b0 == 0)), bool(np.all(b1 == 0)), bool(np.all(b2 == 0)),
    )

    w1_bf = W1.astype(BF16)
    w2_bf = W2.astype(BF16)
    b0r = np.broadcast_to(b0, (128, H)).copy()
    b1r = np.broadcast_to(b1, (128, H)).copy()
    b2r = np.broadcast_to(b2, (128, O)).copy()
    iotaf = np.tile(np.arange(BLK, dtype=np.float32), (128, 1))

    in_maps = []
    for c in range(NCORES):
        pc = percore[c]
        in_maps.append({
            "w1": w1_bf, "w2": w2_bf,
            "b0r": b0r, "b1r": b1r, "b2r": b2r,
            "gidx12": pc["gidx12"],
            "offs12": pc["offs12"],
            "offs0": pc["offs0"],
            "msg0": pc["msg0"],
            "iotaf": iotaf,
        })

    if int(os.environ.get("BASS_GNN_SIM", "0")):
        import concourse.bass_interp as bass_interp
        sim = bass_interp.MultiCoreSim(nc, NCORES)
        for c in range(NCORES):
            for name, arr in in_maps[c].items():
                sim.cores[c].tensor(name)[:] = arr
        sim.simulate()
        outs = [np.asarray(sim.cores[c].mem_tensor("out"))
                for c in range(NCORES)]
        return np.concatenate(outs, axis=0).astype(np.float32)

    trace = bool(int(os.environ.get("BASS_GNN_TRACE", "0")))
    if trace:
        _install_ntff_shim()
    res = bass_utils.run_bass_kernel_spmd(
        nc, in_maps, core_ids=list(range(NCORES)), trace=trace,
    )
    if trace and res.exec_time_ns is not None:
        print(f"HW exec time: {res.exec_time_ns} ns")
        kernel.last_exec_time_ns = res.exec_time_ns
    kernel.last_result = res
    kernel.last_nc = nc
    out = np.concatenate([res.results[c]["out"] for c in range(NCORES)],
                         axis=0)
    return out.astype(np.float32)
